# revision 65
# baseline (speedup 1.0000x reference)
"""DINOLoss Trainium2 Bass kernel — 8-core batch-sharded SPMD (v5).

Decomposition (validated vs reference in numpy, rel err ~1e-6):
  loss = [10*(accDsum - dotCSg_raw + 2*dotCS_raw) + (Cn-1)*(Mg - 2*K_tot)] / 1152
where per-core partials (all-reduced: TP/SS/Sg vectors in d-order + 6 scalars):
  accDsum = sum_j (1/Z_j) * sum_d e_j[d] * (s_raw[global j, d] - Ssum_{b(j)}[d])
  e_j = exp(25*t_j - 110)  (unnormalized teacher exp, bf16), Z_j = row sum
  TP[d] = sum_j e_j[d]/Z_j ;  SS = sum of student rows ; Sg = global rows
  Mg = Lg' + 128*45 ; K_tot = Lsum' + 640*45 ; Lsum' = sum_r ln sum_d exp(10 s - 45)
  dotCS_raw  = 0.9*dot(center,SSg) + (1/1280)*dot(TPg,SSg) ; same for Sg
  Cn = 0.9*sum(center) + 0.1

v5 vs v4 changes (perf):
  - single merged matmul psD = selD^T @ sbufF computes (global_j - Ssum_b) AND
    SS/Sg rows in PSUM -> kills sgdm re-read (4MiB + 64 DMAs), scrB, ssum copy
  - one stt per m: accD (= dotTPS - dotTS partials), scaled by 1/Z at the end
  - TP via 32 [128,4]@[128,512] matmuls with runtime invb weights -> d-order
    staging, no LT permutation, no post-pass transposes
  - student load: 1 DMA per m ; teacher: 16 DMAs ; staging on Pool SWDGE

Layouts (per core, d = k4*16384 + m*1024 + q):
  teacher spread: [128, 16384] bf16, partition 32*k4 + j (j<16 live)
  student m-tile: [80, 4096] f32r, free = k4*1024 + q
  psD chunk m at psum [128, 1024]: rows 32k4+j = global_j - Ssum_b(j),
      row 16 = SS, 17 = Sg (rows 18-31 zero)
  stage_in: [TP (d-order) | SS (d-order) | Sg (d-order) | 6 scalars]
"""

import sys, os
sys.path.insert(0, "/opt/trn_rl_repo")

import numpy as np
import ml_dtypes

import concourse.bass as bass
import concourse.bacc as bacc
import concourse.tile as tile
import concourse.mybir as mybir
from concourse.bass_utils import run_bass_kernel_spmd

F32 = mybir.dt.float32
F32R = mybir.dt.float32r
BF16 = mybir.dt.bfloat16
AF = mybir.ActivationFunctionType
ALU = mybir.AluOpType
AX = mybir.AxisListType

NCORES = 8
B, G, T = 64, 2, 10
D = 65536
P_S, P_T = 80, 16
SCALE_S, SHIFT_S = 10.0, 45.0
SCALE_T, SHIFT_T = 25.0, 110.0
C = 1024
TDEAD = 5.0                # dead teacher rows filler (exp(25*5-110)=e^15, safe)


def _consts():
    # selD [80, 4*128]: for k4-block, col 32*k4+j (j<16): +1 at global row of
    # j minus 1 on all 10 rows of batch b(j); col 32*k4+16: SS (+1 all rows);
    # col 32*k4+17: Sg (+1 global rows)
    sel1 = np.zeros((P_S, 32), np.float32)
    for j in range(16):
        b, t0 = j // 2, j % 2
        sel1[10 * b:10 * b + 10, j] -= 1.0
        sel1[10 * b + t0, j] += 1.0
    sel1[:, 16] = 1.0
    for b in range(8):
        sel1[10 * b:10 * b + 2, 17] = 1.0
    selD = np.zeros((P_S, 4, 128), np.float32)
    for k4 in range(4):
        selD[:, k4, 32 * k4:32 * k4 + 32] = sel1
    selD = selD.reshape(P_S, 512)
    # selK4 [128, 4] ones mask: row 32*k4+j (j<16) -> col k4 (TP weights base)
    selK4 = np.zeros((128, 4), np.float32)
    for k4 in range(4):
        selK4[32 * k4:32 * k4 + 16, k4] = 1.0
    # finW [128, 4]: col0 student rows (Lsum), col1 global rows (Lg),
    # col2 teacher-live rows (accDsum), col3 all ones (post reduction)
    finW = np.zeros((128, 4), np.float32)
    finW[:P_S, 0] = 1.0
    for b in range(8):
        finW[10 * b:10 * b + 2, 1] = 1.0
    for k4 in range(4):
        finW[32 * k4:32 * k4 + 16, 2] = 1.0
    finW[:, 3] = 1.0
    # Wz [128, 16]: gather k4-group partials of teacher row j -> out part j
    Wz = np.zeros((128, 16), np.float32)
    for k4 in range(4):
        for j in range(16):
            Wz[32 * k4 + j, j] = 1.0
    # Wb [16, 128]: broadcast inv[j] -> out partitions 32*k4+j
    Wb = np.zeros((16, 128), np.float32)
    for k4 in range(4):
        for j in range(16):
            Wb[j, 32 * k4 + j] = 1.0
    return selD, selK4.astype(ml_dtypes.bfloat16), finW, Wz, Wb


def build(nc, n_m=16, do_teacher=True, do_main=True, do_coll=True, do_post=True,
          repeat=1, sim_safe=False, do_exp=True, do_mm=True, do_stt=True,
          do_stage=True, do_tp=True):
    d_total = D
    TB = d_total // 4
    TS = TB // 4

    student = nc.dram_tensor("student_shard", [P_S, d_total], F32, kind="ExternalInput")
    teacher = nc.dram_tensor("teacher_shard", [P_T, d_total], F32, kind="ExternalInput")
    center = nc.dram_tensor("center_full", [1, d_total], F32, kind="ExternalInput")
    out_d = nc.dram_tensor("loss", [1, 1], F32, kind="ExternalOutput")

    selD_np, selK4_np, finW_np, Wz_np, Wb_np = _consts()
    selD_d = nc.inline_tensor(selD_np, "selD_c")
    selK4_d = nc.inline_tensor(np.ascontiguousarray(selK4_np), "selK4_c")
    finW_d = nc.inline_tensor(finW_np, "finW_c")
    Wz_d = nc.inline_tensor(Wz_np, "Wz_c")
    Wb_d = nc.inline_tensor(Wb_np, "Wb_c")

    SC_OFF = 3 * d_total
    STAGE = SC_OFF + 8

    with tile.TileContext(nc) as tc:
        with (
            tc.tile_pool(name="const", bufs=1) as cpool,
            tc.tile_pool(name="acc", bufs=1) as apool,
            tc.tile_pool(name="stu", bufs=3) as spool,
            tc.tile_pool(name="traw", bufs=2) as trawpool,
            tc.tile_pool(name="expo", bufs=1) as epool,
            tc.tile_pool(name="scr", bufs=2) as scrpool,
            tc.tile_pool(name="post", bufs=1) as ppool,
            tc.tile_pool(name="psum", bufs=1, space=bass.MemorySpace.PSUM) as psp,
            tc.tile_pool(name="dram", bufs=1, space="DRAM") as dpool,
        ):
            selD_sb = cpool.tile([P_S, 512], F32R)
            nc.sync.dma_start(selD_sb[:], selD_d.ap().bitcast(F32R))
            selK4_sb = cpool.tile([128, 4], BF16)
            nc.sync.dma_start(selK4_sb[:], selK4_d.ap())
            finW_sb = cpool.tile([128, 4], F32)
            nc.sync.dma_start(finW_sb[:], finW_d.ap())
            Wz_sb = cpool.tile([128, 16], F32)
            nc.sync.dma_start(Wz_sb[:], Wz_d.ap())
            Wb_sb = cpool.tile([16, 128], F32)
            nc.sync.dma_start(Wb_sb[:], Wb_d.ap())
            biasS = cpool.tile([128, 1], F32)
            nc.gpsimd.memset(biasS[:], -SHIFT_S)
            biasT = cpool.tile([128, 1], F32)
            nc.gpsimd.memset(biasT[:], -SHIFT_T)

            sacc = apool.tile([P_S, 16], F32)
            accD = apool.tile([128, 16], F32)
            if not do_exp:
                nc.gpsimd.memset(sacc[:], 1.0)
            if not (do_mm and do_stt):
                nc.gpsimd.memset(accD[:], 1.0)
            accDn = apool.tile([128, 16], F32)
            finacc = apool.tile([128, 2], F32)
            nc.gpsimd.memset(finacc[:], 0.0)
            tacc4 = apool.tile([128, 4], F32)
            invb = apool.tile([128, 1], F32)
            w4_dyn = apool.tile([128, 4], BF16)
            er_t = apool.tile([P_S, 1], F32)
            tp_sp = apool.tile([128, TB], BF16)

            stage_in = dpool.tile([STAGE], F32)
            stage_out = dpool.tile([STAGE], F32)

            sap = student.ap()
            tap = teacher.ap()

            if do_teacher and not sim_safe:
                # pre-set dead teacher partitions once; the two traw buffers
                # rotate evenly (4 tile() calls/iter) and DMAs only overwrite
                # live rows, so TDEAD persists across loop iterations
                for _ in range(2):
                    traw_init = trawpool.tile([128, TS], F32, tag="traw4")
                    nc.gpsimd.memset(traw_init[:], TDEAD)

            import contextlib
            loop_cm = tc.For_i(0, repeat, 1) if repeat > 1 else contextlib.nullcontext()
            with loop_cm:
                # ---------------- teacher phase (4 col slices) ----------------
                if not do_teacher:
                    nc.gpsimd.memset(tp_sp[:], 0.001)
                    nc.gpsimd.memset(invb[:], 1.0)
                    nc.gpsimd.memset(w4_dyn[:], 0.001)
                src = sap.rearrange("r (k mm q) -> r k mm q", k=4, mm=16, q=C)
                pre_stu = {}

                def stu_dma(m):
                    t = spool.tile([P_S, 4 * C], F32R, tag="sbufF")
                    nc.sync.dma_start(
                        t[:].rearrange("r (k q) -> r k q", k=4),
                        src[:, :, m, :].bitcast(F32R))
                    return t

                for s in range(4) if do_teacher else []:
                    traw4 = trawpool.tile([128, TS], F32, tag="traw4")
                    if sim_safe:
                        nc.gpsimd.memset(traw4[:], TDEAD)
                    for k4 in range(4):
                        nc.sync.dma_start(
                            traw4[32 * k4:32 * k4 + 16, :],
                            tap[:, k4 * TB + s * TS:k4 * TB + (s + 1) * TS])
                    if do_main and s < 3:
                        pre_stu[s] = stu_dma(s)
                    if do_exp:
                        nc.scalar.activation(tp_sp[:, s * TS:(s + 1) * TS], traw4[:],
                                             AF.Exp, bias=biasT[:], scale=SCALE_T,
                                             accum_out=tacc4[:, s:s + 1])
                if do_teacher and not do_exp:
                    nc.gpsimd.memset(tacc4[:], 1.0)
                if do_teacher:
                    # Z_j = sum_k4 sum_s tacc4[32*k4+j, s] via PE gather;
                    # 1/Z broadcast back to partitions 32*k4+j via PE. No DMAs.
                    psS1 = psp.tile([128, 16], F32, tag="psmall")
                    nc.tensor.matmul(psS1[0:16, 0:4], Wz_sb[:], tacc4[:],
                                     start=True, stop=True)
                    zrow = apool.tile([16, 1], F32)
                    nc.vector.reduce_sum(zrow[:], psS1[0:16, 0:4], axis=AX.X)
                    invj = apool.tile([16, 1], F32)
                    nc.vector.reciprocal(invj[:], zrow[:])
                    nc.tensor.matmul(psS1[:, 4:5], Wb_sb[:], invj[:],
                                     start=True, stop=True)
                    nc.vector.tensor_copy(invb[:], psS1[:, 4:5])
                    nc.vector.tensor_scalar_mul(w4_dyn[:], selK4_sb[:], invb[:])

                def tp_block():
                    # TP in d-order: 32 [128,4]@[128,512] matmuls; chunk
                    # g = 8r + 2b + h lands at psum partitions 32b+k4 (legal
                    # tile_position col bases 0/32/64/96), cols 512h. 4 rounds
                    # of a [128,1024] psum tile -> 4 copies -> 1 DMA.
                    tp_sb = apool.tile([128, 4096], F32)
                    for r in range(4):
                        psU = psp.tile([128, C], F32, tag="psD", bufs=3)
                        for b in range(4):
                            for h in range(2):
                                g = 8 * r + 2 * b + h
                                nc.tensor.matmul(
                                    psU[32 * b:32 * b + 4, 512 * h:512 * h + 512],
                                    w4_dyn[:], tp_sp[:, 512 * g:512 * (g + 1)],
                                    start=True, stop=True,
                                    tile_position=(0, 32 * b))
                        if sim_safe:
                            for b in range(4):
                                nc.vector.tensor_copy(
                                    tp_sb[32 * b:32 * b + 4, C * r:C * (r + 1)],
                                    psU[32 * b:32 * b + 4, :])
                        else:
                            nc.vector.tensor_copy(tp_sb[:, C * r:C * (r + 1)], psU[:])
                    tp_dst = stage_in[0:d_total].rearrange(
                        "(k r b h q) -> k r b h q", k=4, r=4, b=4, h=2, q=512)
                    for b in range(4):
                        nc.gpsimd.dma_start(
                            tp_dst[:, :, b],
                            tp_sb[32 * b:32 * b + 4, :].rearrange(
                                "p (r h q) -> p r h q", r=4, h=2, q=512))

                # ---------------- main pass (16 m-blocks) ----------------
                if do_teacher and do_tp and not do_main:
                    tp_block()
                for m in range(n_m) if do_main else []:
                    if m == 3 and do_teacher and do_tp:
                        # emit TP work here so it overlaps student DMA instead
                        # of blocking the PE/DVE queue heads at the loop start
                        tp_block()
                    if m in pre_stu:
                        sbufF = pre_stu[m]
                    else:
                        sbufF = stu_dma(m)
                    if do_exp:
                        exp_s = epool.tile([P_S, 4 * C], BF16)
                        nc.scalar.activation(exp_s[:], sbufF[:].bitcast(F32), AF.Exp,
                                             bias=biasS[0:P_S], scale=SCALE_S,
                                             accum_out=sacc[:, m:m + 1])
                    if not do_mm:
                        continue
                    psD = psp.tile([128, C], F32, tag="psD", bufs=3)
                    for h in range(2):
                        for k4 in range(4):
                            c0 = k4 * C + 512 * h
                            nc.tensor.matmul(
                                psD[:, 512 * h:512 * h + 512],
                                selD_sb[:, 128 * k4:128 * (k4 + 1)],
                                sbufF[:, c0:c0 + 512],
                                start=(k4 == 0), stop=(k4 == 3))
                    if do_stt:
                        scr = scrpool.tile([128, C], BF16, tag="scr")
                        nc.vector.scalar_tensor_tensor(
                            scr[:], tp_sp[:, m * C:(m + 1) * C], 1.0, psD[:],
                            ALU.mult, ALU.mult, accum_out=accD[:, m:m + 1])
                    if not do_stage:
                        continue
                    ssum = scrpool.tile([128, C], F32, tag="ssum")
                    nc.vector.tensor_copy(ssum[:], psD[:])
                    # stage SS (row 16) and Sg (row 17) in d-order
                    for (row, voff) in ((16, d_total), (17, 2 * d_total)):
                        dst = stage_in[voff:voff + d_total].rearrange(
                            "(k mm q) -> k mm q", k=4, mm=16, q=C)
                        if sim_safe:
                            for jj in range(4):
                                nc.gpsimd.dma_start(
                                    dst[jj, m, :],
                                    ssum[32 * jj + row:32 * jj + row + 1, :])
                        else:
                            nc.gpsimd.dma_start(
                                dst[:, m, :],
                                ssum[:].rearrange("(jj i) q -> jj i q", i=32)[:, row, :])

                # ---------------- finals ----------------
                if do_main:
                    nc.vector.reduce_sum(er_t[:], sacc[:], axis=AX.X)
                    nc.scalar.activation(finacc[0:P_S, 0:1], er_t[:], AF.Ln)
                    nc.vector.tensor_scalar_mul(accDn[:], accD[:], invb[:])
                    nc.vector.reduce_sum(finacc[:, 1:2], accDn[:], axis=AX.X)
                else:
                    nc.gpsimd.memset(finacc[:], 0.001)
                psfin = psp.tile([128, 16], F32, tag="psmall")
                nc.tensor.matmul(psfin[0:3, 6:8], finW_sb[:, 0:3], finacc[:],
                                 start=True, stop=True)
                scl = ppool.tile([3, 2], F32)
                nc.vector.tensor_copy(scl[:], psfin[0:3, 6:8])
                nc.gpsimd.dma_start(
                    stage_in[SC_OFF:SC_OFF + 6].rearrange("(a b) -> a b", b=2), scl[:])

            # ---------------- all-reduce ----------------
            if do_coll:
                nc.gpsimd.collective_compute(
                    "AllReduce", ALU.add,
                    replica_groups=[list(range(NCORES))],
                    ins=[stage_in[:].opt()], outs=[stage_out[:].opt()])
            else:
                nc.sync.dma_start(stage_out[:], stage_in[:])

            # ---------------- post pass (all vectors in d-order) ----------------
            if do_post:
                PQ = d_total // 128
                TPg = ppool.tile([128, PQ], F32)
                nc.sync.dma_start(TPg[:], stage_out[0:d_total].rearrange("(p q) -> p q", p=128))
                SSg = ppool.tile([128, PQ], F32)
                nc.sync.dma_start(SSg[:], stage_out[d_total:2 * d_total].rearrange("(p q) -> p q", p=128))
                Sgg = ppool.tile([128, PQ], F32)
                nc.sync.dma_start(Sgg[:], stage_out[2 * d_total:3 * d_total].rearrange("(p q) -> p q", p=128))
                cen = ppool.tile([128, PQ], F32)
                nc.sync.dma_start(cen[:], center.ap()[0, :].rearrange("(p q) -> p q", p=128))
                sc_sb = ppool.tile([1, 6], F32)
                nc.sync.dma_start(sc_sb[:], stage_out[SC_OFF:SC_OFF + 6])

                # fin2 cols: 0 dot(cen,SSg), 1 dot(TPg,SSg), 2 dot(cen,Sgg),
                # 3 dot(TPg,Sgg), 4 sum(cen)
                fin2 = ppool.tile([128, 5], F32)
                nc.gpsimd.memset(fin2[:], 0.0)
                scrP = ppool.tile([128, PQ], BF16)
                for (vec, c1, c2) in ((SSg, 0, 1), (Sgg, 2, 3)):
                    nc.vector.scalar_tensor_tensor(
                        scrP[:], cen[:], 1.0, vec[:], ALU.mult, ALU.mult,
                        accum_out=fin2[:, c1:c1 + 1])
                    nc.vector.scalar_tensor_tensor(
                        scrP[:], TPg[:], 1.0, vec[:], ALU.mult, ALU.mult,
                        accum_out=fin2[:, c2:c2 + 1])
                nc.vector.reduce_sum(fin2[:, 4:5], cen[:], axis=AX.X)
                psf2 = psp.tile([128, 16], F32, tag="psmall")
                nc.tensor.matmul(psf2[0:1, 8:13], finW_sb[:, 3:4], fin2[:],
                                 start=True, stop=True)
                f2 = ppool.tile([1, 5], F32)
                nc.vector.tensor_copy(f2[:], psf2[0:1, 8:13])

                # scalar arithmetic on partition 0
                fs = ppool.tile([1, 16], F32)
                n_rows_s = NCORES * P_S
                n_rows_g = NCORES * P_T
                # dotCS_raw = 0.9*f2[0] + (1/1280)*f2[1]; dotCSg same on 2,3
                nc.vector.tensor_scalar_mul(fs[:, 0:1], f2[:, 0:1], 0.9)
                nc.vector.scalar_tensor_tensor(
                    fs[:, 1:2], f2[:, 1:2], 1.0 / 1280.0, fs[:, 0:1],
                    ALU.mult, ALU.add)                      # dotCS_raw
                nc.vector.tensor_scalar_mul(fs[:, 2:3], f2[:, 2:3], 0.9)
                nc.vector.scalar_tensor_tensor(
                    fs[:, 3:4], f2[:, 3:4], 1.0 / 1280.0, fs[:, 2:3],
                    ALU.mult, ALU.add)                      # dotCSg_raw
                # Cn = 0.9*sum(cen) + 0.1
                nc.vector.tensor_scalar(fs[:, 4:5], f2[:, 4:5], 0.9, 0.1,
                                        ALU.mult, ALU.add)
                # t3 = accDsum - dotCSg + 2*dotCS
                nc.vector.tensor_scalar_mul(fs[:, 5:6], fs[:, 1:2], 2.0)
                nc.vector.tensor_tensor(fs[:, 6:7], fs[:, 5:6], fs[:, 3:4],
                                        ALU.subtract)       # 2*dotCS - dotCSg
                nc.vector.tensor_tensor(fs[:, 7:8], sc_sb[:, 5:6], fs[:, 6:7],
                                        ALU.add)            # t3
                # cn1 = Cn - 1
                nc.vector.tensor_scalar_add(fs[:, 8:9], fs[:, 4:5], -1.0)
                # m2k = (Lg' + n_g*SHIFT) - 2*(Lsum' + n_s*SHIFT)
                nc.vector.tensor_scalar_mul(fs[:, 9:10], sc_sb[:, 0:1], 2.0)
                nc.vector.tensor_tensor(fs[:, 10:11], sc_sb[:, 2:3], fs[:, 9:10],
                                        ALU.subtract)
                nc.vector.tensor_scalar_add(
                    fs[:, 11:12], fs[:, 10:11],
                    float(n_rows_g * SHIFT_S - 2 * n_rows_s * SHIFT_S))
                # loss = (10*t3 + cn1*m2k) / 1152
                nc.vector.tensor_tensor(fs[:, 12:13], fs[:, 8:9], fs[:, 11:12],
                                        ALU.mult)
                nc.vector.tensor_scalar_mul(fs[:, 13:14], fs[:, 7:8], SCALE_S)
                nc.vector.tensor_tensor(fs[:, 14:15], fs[:, 13:14], fs[:, 12:13],
                                        ALU.add)
                nc.vector.tensor_scalar_mul(fs[:, 15:16], fs[:, 14:15],
                                            1.0 / float(B * G * (T - 1)))
                nc.sync.dma_start(out_d.ap(), fs[:, 15:16])
            else:
                dummy = ppool.tile([1, 1], F32)
                nc.sync.dma_start(dummy[:], stage_out[0:1])
                nc.sync.dma_start(out_d.ap(), dummy[:])
    nc.compile()
    return nc


_CACHE = {}


def _get_nc():
    if "nc" not in _CACHE:
        nc = bacc.Bacc("TRN2", target_bir_lowering=False, debug=False,
                       enable_asserts=False, num_devices=NCORES)
        _CACHE["nc"] = build(nc)
    return _CACHE["nc"]


def kernel(student_output, teacher_output, center, batch_size=64, epoch=0):
    nc = _get_nc()
    student = np.ascontiguousarray(np.asarray(student_output, dtype=np.float32))
    teacher = np.ascontiguousarray(np.asarray(teacher_output, dtype=np.float32))
    cen = np.ascontiguousarray(np.asarray(center, dtype=np.float32))
    in_maps = []
    for c in range(NCORES):
        in_maps.append({
            "student_shard": np.ascontiguousarray(student[P_S * c:P_S * (c + 1)]),
            "teacher_shard": np.ascontiguousarray(teacher[P_T * c:P_T * (c + 1)]),
            "center_full": cen,
        })
    res = run_bass_kernel_spmd(nc, in_maps, core_ids=list(range(NCORES)))
    _CACHE["last_result"] = res
    return np.asarray(res.results[0]["loss"], np.float32).reshape(1)


if __name__ == "__main__":
    import reference
    inputs = reference.setup_inputs()
    expected = np.array(reference.reference(**inputs))
    actual = kernel(**{k: np.asarray(v) for k, v in inputs.items()})
    rel = abs(actual[0] - expected[0]) / abs(expected[0])
    print("expected", expected, "actual", actual, "rel", rel)
